# revision 42
# baseline (speedup 1.0000x reference)
"""Low-rank cross-attention on 8 Trainium2 NeuronCores (Bass/Tile).

Problem: out = (softmax((tgt@Wq.T)(memory@Wk.T).T / sqrt(r)) @ (memory@Wv.T)) @ Wo.T
Shapes: tgt/memory [4, 2048, 1024], r=128, d_model=1024.

Sharding: core c in 0..7 handles batch b=c//2 and query-half h=c%2
(1024 query tokens) against the full 2048-token memory of its batch.
No collectives.

KEY ALGEBRAIC FUSION: with dropout = identity, the value/output
projections re-associate:
    out = (softmax(S) @ (mem @ Wv.T)) @ Wo.T
        = (softmax(S) @ mem) @ (Wo @ Wv).T
W2 = Wo @ Wv is precomputed (free) on the host, so the device computes
    G[d,t] = sum_s exp[s,t] * mem[s,d]        (contract s, like attn@mem)
    out    = (1/rowsum) * G.T @ W2.T          (contract d)
This replaces v-proj (S*D*E) + attn@v (E*S*T) + out-proj (T*E*O) MACs
with G (D*S*T) + out (T*D*O): 2.1 GMAC saved per core (~55us of PE
time), because the value projection no longer scales with the full
memory length.  (The low-rank q/k path stays factored - that IS the
cheap path.)

Layouts: every matmul has its contraction dim on the SBUF partition
axis.  mem is fed in BOTH layouts (memT d-major for the k-projection,
memS s-major for G) - DMA has headroom, PE does not.
  qT [r,T]   = WqT.T @ tgtT             (contract d)
  kT [r,S]   = WkT.T @ memT             (contract d)
  exT[S,Tq]  = exp(scale * kT_s.T @ qT) (contract r, single MM)
  G  [d,Tq]  = memS_s.T @ exT           (contract s)
  out[Tq,o]  = G_t.T @ W2T              (contract d)

All Phase-B matmuls stream 512 columns so LDWEIGHTS fully hides under
the moving-data stream.  Softmax row-sums stay OFF the PE: the Vector
engine accumulates exp tiles into a fp32 acc [128,512]; four tiny fp32
matmuls (lhsT = acc 128-col block, rhs = ones) then yield the
TRANSPOSED per-query sums [128,1] directly in PSUM, so the reciprocal
needs no DRAM round-trip.  The division is folded into the final
PSUM->SBUF scaling.  Logits are bounded (|x| < ~15) so exp is fp32-safe
with no max subtraction.  All matmuls run in bf16 (inputs cast
host-side).
"""

import ml_dtypes
import numpy as np

import concourse.bass as bass
import concourse.mybir as mybir
import concourse.tile as tile
from concourse.bacc import Bacc
from concourse.bass_utils import run_bass_kernel_spmd

FP = mybir.dt.float32
BF = mybir.dt.bfloat16
ts = bass.ts

B = 4
T_FULL = 2048
D = 1024
R = 128
S = 2048
O = 1024
T = 1024            # per-core query tokens (half of T_FULL)
P = 128
SCALE = 1.0 / np.sqrt(128.0)

KD = D // P         # 8 contraction tiles over d
NS = S // P         # 16 key/memory tiles
ND = D // P         # 8 G feature tiles
TQ = 512            # query-column strip processed per attention pass
NH = T // TQ        # 2 halves
NTT = TQ // P       # 4 query 128-blocks per half

# Set by test harness to enable NTFF profiling; LAST_RESULT holds the
# BassKernelResults of the most recent kernel() call.
TRACE = False
LAST_RESULT = None
_PROG = None


def _build_program(linearize=False):
    # Bacc (not raw Bass): its finalize() runs move_matmul_waits_to_ldweights
    # + generate_event_semaphores, which split multi-sem waits down to the
    # one-wait-per-instruction limit of the TRN2 ISA. Raw Bass trips
    # walrus's "Too many sync wait commands" codegen error.
    nc = Bacc()

    tgtT_d = nc.dram_tensor("tgtT", [D, T], BF, kind="ExternalInput")
    memT_d = nc.dram_tensor("memT", [D, S], BF, kind="ExternalInput")
    memS_d = nc.dram_tensor("memS", [S, D], BF, kind="ExternalInput")
    # wq/wk are host-packed with their 8 d-tiles side by side so each loads
    # as ONE 2KB-row DMA instead of eight 256B-row ones (~1.5us at start)
    wq_d = nc.dram_tensor("WqP", [P, KD * R], BF, kind="ExternalInput")
    wk_d = nc.dram_tensor("WkP", [P, KD * R], BF, kind="ExternalInput")
    w2_d = nc.dram_tensor("W2T", [D, O], BF, kind="ExternalInput")
    # bf16 output: halves the tail DMA drain after the last matmul; the
    # ~0.2% quantization is well inside the error budget.
    out_d = nc.dram_tensor("out", [T, O], BF, kind="ExternalOutput")

    Exp = mybir.ActivationFunctionType.Exp

    with tile.TileContext(nc, linearize=linearize) as tc:
        with tc.tile_pool(name="perm", bufs=1) as perm, \
             tc.tile_pool(name="expp", bufs=1) as expp, \
             tc.tile_pool(name="accp", bufs=1) as accp, \
             tc.tile_pool(name="gsb", bufs=1) as gsb, \
             tc.tile_pool(name="rcsb", bufs=1) as rcsb, \
             tc.tile_pool(name="outp", bufs=3) as outp:
            qT = perm.tile([P, T], BF, tag="qT")
            kT = perm.tile([P, S], BF, tag="kT")
            ones_f = perm.tile([P, 1], FP, tag="ones_f")
            nc.vector.memset(ones_f, 1.0)

            memT = [perm.tile([P, S], BF, tag=f"m{k}", name=f"m{k}") for k in range(KD)]
            memS = [perm.tile([P, D], BF, tag=f"s{m}", name=f"s{m}") for m in range(NS)]
            tgt = [perm.tile([P, T], BF, tag=f"t{k}", name=f"t{k}") for k in range(KD)]
            wkP = perm.tile([P, KD * R], BF, tag="wkP")
            wqP = perm.tile([P, KD * R], BF, tag="wqP")
            wk = [wkP[:, ts(k, R)] for k in range(KD)]
            wq = [wqP[:, ts(k, R)] for k in range(KD)]
            w2 = [perm.tile([P, O], BF, tag=f"w2{k}", name=f"w2{k}") for k in range(ND)]

            # Input DMA: only TWO hardware DGE queues exist (sync=SP and
            # scalar=Activation), each topping out well under HBM peak, so
            # split every load group across both queues (alternating k) and
            # keep per-partition rows >= 2KB for packet efficiency.  Issue
            # order = consumption order: kT(0,1) -> qT -> kT(2,3) ->
            # scores -> G (memS s ascending) -> out (w2).
            def dma2(i, **kw):
                (nc.sync if i % 2 == 0 else nc.scalar).dma_start(**kw)

            # Early loads (kT/qT deps) split across both queues; everything
            # needed later (memS, w2) goes on the sync queue ONLY - the
            # scalar engine must drain its DMA-trigger instructions before
            # it can run the first exp, so its queue has to stay short.
            nc.sync.dma_start(out=wkP, in_=wk_d[:, :])
            nc.scalar.dma_start(out=wqP, in_=wq_d[:, :])
            # memT in quarter strips, interleaved with tgt and the first
            # memS tiles so every consumer's deadline is met at the ~360
            # GB/s two-queue arrival rate: kT(0) @ ~10, qT(0) @ ~16 (gates
            # the scores/G pipeline), kT strip n (stolen into loopA) @
            # ~21+2n, memS[s] just ahead of G(s).
            def memT_quarter(q):
                for k in range(KD):
                    dma2(k + q, out=memT[k][:, ts(q, TQ)],
                         in_=memT_d[ts(k, P), ts(q, TQ)])

            memT_quarter(0)
            for k in range(KD):
                dma2(k, out=tgt[k], in_=tgtT_d[ts(k, P), :])
            memT_quarter(1)
            nc.sync.dma_start(out=memS[0], in_=memS_d[0:P, :])
            nc.sync.dma_start(out=memS[1], in_=memS_d[P:2 * P, :])
            memT_quarter(2)
            nc.sync.dma_start(out=memS[2], in_=memS_d[2 * P:3 * P, :])
            nc.sync.dma_start(out=memS[3], in_=memS_d[3 * P:4 * P, :])
            # quarter 3 entirely on sync: the scalar engine blocks on its
            # DMA-trigger ring before it can run the first exp, so its
            # queue must stay short; the sync engine has no compute to
            # block.  kt3's steal point moves one iteration later to match.
            for k in range(KD):
                nc.sync.dma_start(out=memT[k][:, 3 * TQ:S],
                                  in_=memT_d[ts(k, P), 3 * TQ:S])
            for m in range(4, NS):
                nc.sync.dma_start(out=memS[m], in_=memS_d[ts(m, P), :])
            for k in range(ND):
                nc.sync.dma_start(out=w2[k], in_=w2_d[ts(k, P), :])

            # PSUM budget (8 banks): one recycled 4-bank pool ("ps", tags
            # p0..p3) serves the Phase-A projection groups, then the G
            # accumulators of both d-passes, the rc matmul output, and the
            # out-proj accumulators (generation WAR chains give the needed
            # ordering) + psc 4 (scores prefetch ring).
            with tc.tile_pool(name="ps", bufs=1, space="PSUM") as psp, \
                 tc.tile_pool(name="psc", bufs=4, space="PSUM") as psc:
                pcount = [0]

                def ps_tile(name):
                    t = psp.tile([P, TQ], FP, tag=f"p{pcount[0] % 4}", name=name)
                    pcount[0] += 1
                    return t

                # ---- Phase A: projections kT(0,1), qT, kT(2,3) ----
                def kT_group(n):
                    ps = ps_tile(f"kt{n}")
                    for k in range(KD):
                        nc.tensor.matmul(ps, wk[k], memT[k][:, ts(n, TQ)],
                                         start=(k == 0), stop=(k == KD - 1))
                    nc.vector.tensor_copy(kT[:, ts(n, TQ)], ps)

                def qT_group(n):
                    ps = ps_tile(f"qt{n}")
                    for k in range(KD):
                        nc.tensor.matmul(ps, wq[k], tgt[k][:, ts(n, TQ)],
                                         start=(k == 0), stop=(k == KD - 1))
                    nc.vector.tensor_copy(qT[:, ts(n, TQ)], ps)

                def kT_group_psc(n):
                    # late kT strips, computed through the scores PSUM ring
                    # so they can be stolen into loopA of half 0 (the main
                    # p-tags hold G accumulators by then)
                    ps = psc.tile([P, TQ], FP, name="sc")
                    for k in range(KD):
                        nc.tensor.matmul(ps, wk[k], memT[k][:, ts(n, TQ)],
                                         start=(k == 0), stop=(k == KD - 1))
                    nc.vector.tensor_copy(kT[:, ts(n, TQ)], ps)

                def qT_group_psc(n):
                    # qT half 1 is only read by half-1 scores; steal it into
                    # loopA of half 0 so nothing waits on the tgt DMAs
                    ps = psc.tile([P, TQ], FP, name="sc")
                    for k in range(KD):
                        nc.tensor.matmul(ps, wq[k], tgt[k][:, ts(n, TQ)],
                                         start=(k == 0), stop=(k == KD - 1))
                    nc.vector.tensor_copy(qT[:, ts(n, TQ)], ps)

                kT_group(0)
                qT_group(0)
                kT_group(1)

                # ---- Phase B: attention + out projection, per 512-col half --
                for h in range(NH):
                    tq = slice(h * TQ, (h + 1) * TQ)
                    ex = [expp.tile([P, TQ], BF, tag=f"ex{s}", name=f"ex{s}")
                          for s in range(NS)]
                    acc = accp.tile([P, TQ], FP, tag="acc")

                    def scores(s, tq=tq, ex=ex, acc=acc):
                        sc = psc.tile([P, TQ], FP)
                        nc.tensor.matmul(sc, kT[:, ts(s, P)], qT[:, tq],
                                         start=True, stop=True)
                        nc.scalar.activation(ex[s], sc, Exp, scale=float(SCALE))
                        if s == 0:
                            nc.vector.tensor_copy(acc, ex[s])
                        else:
                            nc.vector.tensor_add(acc, acc, ex[s])

                    # pass 1: d-blocks 0..3 of G accumulate over all s; the
                    # scores/exp pipeline runs 3 s-tiles ahead (psc is a
                    # 4-deep ring).
                    for s in range(3):
                        scores(s)
                    if h == 0:
                        # fills the exp(0) latency before the first G matmul
                        qT_group_psc(1)
                    g1 = [ps_tile(f"g{h}_{j}") for j in range(4)]
                    for s in range(NS):
                        if s + 3 < NS:
                            scores(s + 3)
                        for e in range(4):
                            nc.tensor.matmul(g1[e][:, :], memS[s][:, ts(e, P)],
                                             ex[s], start=(s == 0),
                                             stop=(s == NS - 1))
                        if h == 0 and s in (4, 7):
                            # kT strips 2/3 stolen into loopA: strip 2 is
                            # first needed by scores(8) at s=5, strip 3 by
                            # scores(12) at s=9.  Emitted AFTER this
                            # iteration's G matmuls so the steal's LDW
                            # issues ~0.9us later - halving its wait on the
                            # memT quarter-strip DMA arrival.
                            kT_group_psc(2 + (s == 7))
                    g_sb = [gsb.tile([P, TQ], BF, tag=f"gs{e}", name=f"gs{e}")
                            for e in range(ND)]
                    for e in range(4):
                        nc.vector.tensor_copy(g_sb[e], g1[e])

                    # pass 2: d-blocks 4..7 (all ex tiles now resident)
                    g2 = [ps_tile(f"g{h}_{j}2") for j in range(4)]
                    for e in range(4):
                        for s in range(NS):
                            nc.tensor.matmul(g2[e][:, :], memS[s][:, ts(e + 4, P)],
                                             ex[s], start=(s == 0),
                                             stop=(s == NS - 1))
                        nc.vector.tensor_copy(g_sb[e + 4], g2[e])

                    # transposed per-query sums: rc_ps[i, tt] =
                    # sum_p acc[p, tt*128+i]; four single-column fp32
                    # matmuls share one PSUM tile (start only on the first
                    # clears the bank's has_written bits).
                    rc_ps = ps_tile(f"rc{h}")
                    for tt in range(NTT):
                        nc.tensor.matmul(rc_ps[:, tt:tt + 1],
                                         acc[:, ts(tt, P)], ones_f,
                                         start=(tt == 0), stop=(tt == NTT - 1))
                    rc = rcsb.tile([P, NTT], FP, tag="rc_sb")
                    nc.vector.reciprocal(rc, rc_ps[:, 0:NTT])

                    # out projection: 8 groups of 8 accumulating matmuls
                    for g in range(NTT * (O // TQ)):
                        tt, oh = divmod(g, O // TQ)
                        po = ps_tile(f"po{h}_{g}")
                        for e in range(ND):
                            nc.tensor.matmul(po, g_sb[e][:, ts(tt, P)],
                                             w2[e][:, ts(oh, TQ)],
                                             start=(e == 0), stop=(e == ND - 1))
                        ob = outp.tile([P, TQ], BF)
                        nc.vector.tensor_scalar_mul(ob, po, rc[:, tt:tt + 1])
                        if h == NH - 1 and g == NTT * (O // TQ) - 1:
                            # split the very last store across both queues
                            # so the post-matmul drain is half as long
                            for c in range(4):
                                dma2(c, out=out_d[ts(h * NTT + tt, P),
                                                  oh * TQ + c * P:
                                                  oh * TQ + (c + 1) * P],
                                     in_=ob[:, ts(c, P)])
                        else:
                            dma2(g, out=out_d[ts(h * NTT + tt, P), ts(oh, TQ)],
                                 in_=ob)
    return nc


def kernel(tgt, memory, Wq, Wk, Wv, Wo):
    """8-way data-parallel (batch x query-half) low-rank cross-attention
    on the 8 NeuronCores via the Bass/Tile kernel above."""
    global LAST_RESULT, _PROG

    tgt = np.asarray(tgt, dtype=np.float32)
    memory = np.asarray(memory, dtype=np.float32)
    BFnp = ml_dtypes.bfloat16

    # pack the 8 d-tiles of WqT/WkT side by side: wP[p, k*R+j] = W.T[k*P+p, j]
    def pack(w):
        wt = np.asarray(w, np.float32).T.reshape(KD, P, R)
        return np.ascontiguousarray(
            wt.transpose(1, 0, 2).reshape(P, KD * R)).astype(BFnp)

    wqP = pack(Wq)
    wkP = pack(Wk)
    # value/output projections fused on the host: W2 = Wo @ Wv, fed
    # transposed so the contraction dim (d) lands on SBUF partitions.
    w2T = np.ascontiguousarray(
        (np.asarray(Wo, np.float32) @ np.asarray(Wv, np.float32)).T
    ).astype(BFnp)

    in_maps = []
    for c in range(8):
        b, h = divmod(c, 2)
        tgtT = np.ascontiguousarray(
            tgt[b, h * T:(h + 1) * T, :].T).astype(BFnp)        # [D, T]
        memT = np.ascontiguousarray(memory[b].T).astype(BFnp)   # [D, S]
        memS = np.ascontiguousarray(memory[b]).astype(BFnp)     # [S, D]
        in_maps.append({"tgtT": tgtT, "memT": memT, "memS": memS,
                        "WqP": wqP, "WkP": wkP, "W2T": w2T})

    if _PROG is None:
        _PROG = _build_program()
        # Bacc defers register allocation to finalize(); the bass_exec
        # lowering serializes the module as-is, so finalize here or walrus
        # sees reg_id=-1 ("Reg has not been allocated yet").
        _PROG.finalize()
    res = run_bass_kernel_spmd(_PROG, in_maps, core_ids=list(range(8)),
                               trace=TRACE)
    LAST_RESULT = res

    out = np.empty((B, T_FULL, O), dtype=np.float32)
    for c in range(8):
        b, h = divmod(c, 2)
        out[b, h * T:(h + 1) * T, :] = res.results[c]["out"].astype(np.float32)
    return out
